# revision 1
# baseline (speedup 1.0000x reference)
"""AttentiveStatisticsPooling Trainium2 kernel (8 NeuronCores, batch-sharded).

Reference computation (B=32, C=1536, T=2000):
    a    = einsum('bct,c->bt', x, w) + cb          # 1x1 conv -> [B,T]
    a    = BN(a)  (batch stats over all B*T, biased var)    # syncBN via AllReduce
    attn = softmax(tanh(a), axis=T)
    mean = einsum('bct,bt->bc', x, attn)
    std  = sqrt(clip(E_attn[x^2] - mean^2, 1e-10))
    out  = concat([mean, std], axis=1)             # [B, 2C]

Sharding: batch across 8 cores (4 samples each). BN batch stats are exact:
local (sum a, sum a^2) are AllReduced across cores.  conv_b cancels out of
BN(a) exactly (shift invariance), so it is unused.

Per-core dataflow:
  pass 1: stream x[b] tiles [128c x 2000t]; PE matmul with conv_w replicated
          across 128 output partitions (lhsT [128,128] with identical
          columns) -> a broadcast to all partitions; stats on DVE/ACT.
  mid:    AllReduce 2 scalars; BN affine folded into ACT Tanh (per-partition
          scale/bias APs); ACT Exp with accum -> softmax denominator.
  pass 2: stream x tiles again; DVE tensor_mul y=x*attn, z=y*x; ACT
          Copy-activation with accum_out reduces y and z over T.
"""

import numpy as np

B, C, T = 32, 1536, 2000
NCORES = 8
BSH = B // NCORES          # 4 samples per core
KC = C // 128              # 12 channel chunks
NTC = 4                    # psum T chunks
TC = T // NTC              # 500 (<=512 psum bank)
BN_EPS = 1e-5

_CACHE = {}


def _build():
    import concourse.bacc as bacc
    import concourse.tile as tile
    import concourse.mybir as mybir

    f32 = mybir.dt.float32
    AF = mybir.ActivationFunctionType
    AX = mybir.AxisListType

    nc = bacc.Bacc("TRN2", target_bir_lowering=False, debug=False,
                   enable_asserts=True, num_devices=NCORES)
    x = nc.dram_tensor("x", [BSH, C, T], f32, kind="ExternalInput").ap()
    w = nc.dram_tensor("conv_w", [C], f32, kind="ExternalInput").ap()
    gamma = nc.dram_tensor("bn_gamma", [1], f32, kind="ExternalInput").ap()
    beta = nc.dram_tensor("bn_beta", [1], f32, kind="ExternalInput").ap()
    out = nc.dram_tensor("out", [BSH, 2 * C], f32, kind="ExternalOutput").ap()

    with tile.TileContext(nc) as tc:
        with (
            tc.tile_pool(name="singles", bufs=1) as singles,
            tc.tile_pool(name="xin", bufs=3) as xin,
            tc.tile_pool(name="ypool", bufs=2) as ypool,
            tc.tile_pool(name="zpool", bufs=2) as zpool,
            tc.tile_pool(name="scr", bufs=2) as scrp,
            tc.tile_pool(name="stats", bufs=4) as stats,
            tc.tile_pool(name="psum", bufs=2, space="PSUM") as psum,
            tc.tile_pool(name="dram", bufs=1, space="DRAM") as dram,
        ):
            # ---- setup: conv_w replicated [128, kc, 128] ----
            w_sb = singles.tile([128, KC], f32)
            nc.sync.dma_start(out=w_sb[:], in_=w.rearrange("(kc p) -> p kc", p=128))
            ones = singles.tile([128, 128], f32)
            nc.vector.memset(ones[:], 1.0)
            w_rep = singles.tile([128, KC, 128], f32)
            for kc in range(KC):
                nc.scalar.mul(w_rep[:, kc, :], ones[:], w_sb[:, kc:kc + 1])

            gamma_sb = singles.tile([128, 1], f32)
            nc.gpsimd.dma_start(out=gamma_sb[:], in_=gamma.to_broadcast((128, 1)))
            beta_sb = singles.tile([128, 1], f32)
            nc.gpsimd.dma_start(out=beta_sb[:], in_=beta.to_broadcast((128, 1)))

            # attn (and a before it), broadcast on all 128 partitions
            a_all = singles.tile([128, BSH, T], f32)
            sumsA = singles.tile([128, BSH], f32)
            sumsA2 = singles.tile([128, BSH], f32)

            # ---- pass 1: a[b,t] broadcast over partitions, local stats ----
            for b in range(BSH):
                pa = psum.tile([128, NTC, 512], f32)
                for kc in range(KC):
                    x_t = xin.tile([128, T], f32)
                    nc.sync.dma_start(out=x_t[:], in_=x[b, kc * 128:(kc + 1) * 128, :])
                    for i in range(NTC):
                        nc.tensor.matmul(pa[:, i, 0:TC], w_rep[:, kc, :],
                                         x_t[:, i * TC:(i + 1) * TC],
                                         start=(kc == 0), stop=(kc == KC - 1))
                a_b = a_all[:, b, :].rearrange("p (i n) -> p i n", i=NTC)
                nc.scalar.copy(a_b, pa[:, :, 0:TC])
                nc.vector.reduce_sum(sumsA[:, b:b + 1], a_all[:, b, :], axis=AX.X)
                sq_scr = scrp.tile([128, T], f32, tag="scr")
                nc.scalar.activation(sq_scr[:], a_all[:, b, :], AF.Square,
                                     accum_out=sumsA2[:, b:b + 1])

            # ---- mid: allreduce BN stats, softmax(tanh(bn(a))) ----
            totA = stats.tile([128, 1], f32, tag="small")
            nc.vector.reduce_sum(totA[:], sumsA[:], axis=AX.X)
            totA2 = stats.tile([128, 1], f32, tag="small")
            nc.vector.reduce_sum(totA2[:], sumsA2[:], axis=AX.X)

            cc_in = dram.tile([1, 2], f32)
            cc_out = dram.tile([1, 2], f32)
            nc.sync.dma_start(out=cc_in[0:1, 0:1], in_=totA[0:1, 0:1])
            nc.sync.dma_start(out=cc_in[0:1, 1:2], in_=totA2[0:1, 0:1])
            nc.gpsimd.collective_compute(
                "AllReduce", mybir.AluOpType.add,
                replica_groups=[list(range(NCORES))],
                ins=[cc_in.opt()], outs=[cc_out.opt()])
            g = stats.tile([128, 2], f32, tag="small2")
            nc.gpsimd.dma_start(out=g[:], in_=cc_out.to_broadcast((128, 2)))

            inv_n = 1.0 / float(B * T)
            mu = stats.tile([128, 1], f32, tag="small")
            nc.vector.tensor_scalar_mul(mu[:], g[:, 0:1], inv_n)
            ex2 = stats.tile([128, 1], f32, tag="small")
            nc.vector.tensor_scalar_mul(ex2[:], g[:, 1:2], inv_n)
            m2 = stats.tile([128, 1], f32, tag="small")
            nc.vector.tensor_mul(m2[:], mu[:], mu[:])
            var = stats.tile([128, 1], f32, tag="small")
            nc.vector.tensor_sub(var[:], ex2[:], m2[:])
            vep = stats.tile([128, 1], f32, tag="small")
            nc.vector.tensor_scalar_add(vep[:], var[:], BN_EPS)
            sd = stats.tile([128, 1], f32, tag="small")
            nc.scalar.sqrt(sd[:], vep[:])
            rstd = stats.tile([128, 1], f32, tag="small")
            nc.vector.reciprocal(rstd[:], sd[:])
            scl = singles.tile([128, 1], f32)
            nc.vector.tensor_mul(scl[:], rstd[:], gamma_sb[:])
            msc = stats.tile([128, 1], f32, tag="small")
            nc.vector.tensor_mul(msc[:], mu[:], scl[:])
            bias = singles.tile([128, 1], f32)
            nc.vector.tensor_sub(bias[:], beta_sb[:], msc[:])

            Zb = singles.tile([128, BSH], f32)
            rZ = singles.tile([128, BSH], f32)
            for b in range(BSH):
                z_scr = scrp.tile([128, T], f32, tag="scr")
                nc.scalar.activation(z_scr[:], a_all[:, b, :], AF.Tanh,
                                     bias=bias[:, 0:1], scale=scl[:, 0:1])
                nc.scalar.activation(a_all[:, b, :], z_scr[:], AF.Exp,
                                     accum_out=Zb[:, b:b + 1])
                nc.vector.reciprocal(rZ[:, b:b + 1], Zb[:, b:b + 1])
                nc.vector.tensor_scalar_mul(a_all[:, b, :], a_all[:, b, :],
                                            rZ[:, b:b + 1])

            # ---- pass 2: weighted mean / sqmean over T ----
            dump = singles.tile([128, T], f32)
            for b in range(BSH):
                meanT = stats.tile([128, KC], f32, tag="mean")
                sqT = stats.tile([128, KC], f32, tag="sq")
                for kc in range(KC):
                    x_t = xin.tile([128, T], f32)
                    nc.sync.dma_start(out=x_t[:], in_=x[b, kc * 128:(kc + 1) * 128, :])
                    y = ypool.tile([128, T], f32)
                    nc.vector.tensor_mul(y[:], x_t[:], a_all[:, b, :])
                    nc.scalar.activation(dump[:], y[:], AF.Copy,
                                         accum_out=meanT[:, kc:kc + 1])
                    z2 = zpool.tile([128, T], f32)
                    nc.vector.tensor_mul(z2[:], y[:], x_t[:])
                    nc.scalar.activation(dump[:], z2[:], AF.Copy,
                                         accum_out=sqT[:, kc:kc + 1])
                m2o = stats.tile([128, KC], f32, tag="m2o")
                nc.vector.tensor_mul(m2o[:], meanT[:], meanT[:])
                vo = stats.tile([128, KC], f32, tag="vo")
                nc.vector.tensor_sub(vo[:], sqT[:], m2o[:])
                nc.vector.tensor_scalar_max(vo[:], vo[:], 1e-10)
                stdT = stats.tile([128, KC], f32, tag="std")
                nc.scalar.sqrt(stdT[:], vo[:])
                nc.sync.dma_start(
                    out=out[b:b + 1, 0:C].rearrange("a (kc p) -> p (a kc)", p=128),
                    in_=meanT[:])
                nc.sync.dma_start(
                    out=out[b:b + 1, C:2 * C].rearrange("a (kc p) -> p (a kc)", p=128),
                    in_=stdT[:])
    nc.compile()
    return nc


def _get_nc():
    if "nc" not in _CACHE:
        _CACHE["nc"] = _build()
    return _CACHE["nc"]


def kernel(x, conv_w, conv_b, bn_gamma, bn_beta):
    from concourse.bass_utils import run_bass_kernel_spmd

    x = np.ascontiguousarray(np.asarray(x, dtype=np.float32))
    conv_w = np.asarray(conv_w, dtype=np.float32)
    bn_gamma = np.asarray(bn_gamma, dtype=np.float32)
    bn_beta = np.asarray(bn_beta, dtype=np.float32)

    nc = _get_nc()
    in_maps = [
        {"x": x[i * BSH:(i + 1) * BSH], "conv_w": conv_w,
         "bn_gamma": bn_gamma, "bn_beta": bn_beta}
        for i in range(NCORES)
    ]
    res = run_bass_kernel_spmd(nc, in_maps, core_ids=list(range(NCORES)))
    return np.concatenate([r["out"] for r in res.results], axis=0)


# revision 2
# speedup vs baseline: 8.3351x; 8.3351x over previous
"""AttentiveStatisticsPooling Trainium2 kernel (8 NeuronCores, batch-sharded).

Reference computation (B=32, C=1536, T=2000):
    a    = einsum('bct,c->bt', x, w) + cb          # 1x1 conv -> [B,T]
    a    = BN(a)  (batch stats over all B*T, biased var)    # syncBN via AllReduce
    attn = softmax(tanh(a), axis=T)
    mean = einsum('bct,bt->bc', x, attn)
    std  = sqrt(clip(E_attn[x^2] - mean^2, 1e-10))
    out  = concat([mean, std], axis=1)             # [B, 2C]

Sharding: batch across 8 cores (4 samples each). BN batch stats are exact:
local (sum a, sum a^2) are AllReduced across cores.  conv_b cancels out of
BN(a) exactly (shift invariance), so it is unused.

Per-core dataflow:
  pass 1: stream x[b] tiles [128c x 2000t]; PE matmul with conv_w replicated
          across 128 output partitions (lhsT [128,128] with identical
          columns) -> a broadcast to all partitions; stats on DVE/ACT.
  mid:    AllReduce 2 scalars; BN affine folded into ACT Tanh (per-partition
          scale/bias APs); ACT Exp with accum -> softmax denominator.
  pass 2: stream x tiles again; DVE tensor_mul y=x*attn, z=y*x; ACT
          Copy-activation with accum_out reduces y and z over T.
"""

import numpy as np

B, C, T = 32, 1536, 2000
NCORES = 8
BSH = B // NCORES          # 4 samples per core
KC = C // 128              # 12 channel chunks
NTC = 4                    # psum T chunks
TC = T // NTC              # 500 (<=512 psum bank)
BN_EPS = 1e-5

_CACHE = {}


def _build(nrep=1):
    import concourse.bacc as bacc
    import concourse.tile as tile
    import concourse.mybir as mybir

    f32 = mybir.dt.float32
    AF = mybir.ActivationFunctionType
    AX = mybir.AxisListType

    nc = bacc.Bacc("TRN2", target_bir_lowering=False, debug=False,
                   enable_asserts=True, num_devices=NCORES)
    x = nc.dram_tensor("x", [BSH, C, T], f32, kind="ExternalInput").ap()
    w = nc.dram_tensor("conv_w", [C], f32, kind="ExternalInput").ap()
    gamma = nc.dram_tensor("bn_gamma", [1], f32, kind="ExternalInput").ap()
    beta = nc.dram_tensor("bn_beta", [1], f32, kind="ExternalInput").ap()
    out = nc.dram_tensor("out", [BSH, 2 * C], f32, kind="ExternalOutput").ap()

    with tile.TileContext(nc) as tc:
        with (
            tc.tile_pool(name="singles", bufs=1) as singles,
            tc.tile_pool(name="xin", bufs=4) as xin,
            tc.tile_pool(name="ypool", bufs=2) as ypool,
            tc.tile_pool(name="zpool", bufs=2) as zpool,
            tc.tile_pool(name="scr", bufs=2) as scrp,
            tc.tile_pool(name="stats", bufs=4) as stats,
            tc.tile_pool(name="psum", bufs=2, space="PSUM") as psum,
            tc.tile_pool(name="dram", bufs=2, space="DRAM") as dram,
        ):
            # ---- setup: conv_w replicated [128, kc, 128] ----
            w_sb = singles.tile([128, KC], f32)
            nc.sync.dma_start(out=w_sb[:], in_=w.rearrange("(kc p) -> p kc", p=128))
            ones = singles.tile([128, 128], f32)
            nc.vector.memset(ones[:], 1.0)
            w_rep = singles.tile([128, KC, 128], f32)
            for kc in range(KC):
                nc.scalar.mul(w_rep[:, kc, :], ones[:], w_sb[:, kc:kc + 1])

            gamma_sb = singles.tile([128, 1], f32)
            nc.gpsimd.dma_start(out=gamma_sb[:], in_=gamma.to_broadcast((128, 1)))
            beta_sb = singles.tile([128, 1], f32)
            nc.gpsimd.dma_start(out=beta_sb[:], in_=beta.to_broadcast((128, 1)))

            # attn (and a before it), broadcast on all 128 partitions
            a_all = singles.tile([128, BSH, T], f32)
            sumsA = singles.tile([128, BSH], f32)
            sumsA2 = singles.tile([128, BSH], f32)
            Zb = singles.tile([128, BSH], f32)
            rZ = singles.tile([128, BSH], f32)
            dump = singles.tile([128, T], f32)

            for _rep in range(nrep):
                # ---- pass 1: a[b,t] broadcast over partitions, local stats ----
                for b in range(BSH):
                    pa = psum.tile([128, NTC, 512], f32)
                    for kc in range(KC):
                        x_t = xin.tile([128, T], f32)
                        nc.sync.dma_start(out=x_t[:],
                                          in_=x[b, kc * 128:(kc + 1) * 128, :])
                        for i in range(NTC):
                            nc.tensor.matmul(pa[:, i, 0:TC], w_rep[:, kc, :],
                                             x_t[:, i * TC:(i + 1) * TC],
                                             start=(kc == 0), stop=(kc == KC - 1))
                    a_b = a_all[:, b, :].rearrange("p (i n) -> p i n", i=NTC)
                    nc.scalar.copy(a_b, pa[:, :, 0:TC])
                    nc.vector.reduce_sum(sumsA[:, b:b + 1], a_all[:, b, :], axis=AX.X)
                    sq_scr = scrp.tile([128, T], f32, tag="scr")
                    nc.scalar.activation(sq_scr[:], a_all[:, b, :], AF.Square,
                                         accum_out=sumsA2[:, b:b + 1])

                # ---- mid: allreduce BN stats, softmax(tanh(bn(a))) ----
                totA = stats.tile([128, 1], f32, tag="small")
                nc.vector.reduce_sum(totA[:], sumsA[:], axis=AX.X)
                totA2 = stats.tile([128, 1], f32, tag="small")
                nc.vector.reduce_sum(totA2[:], sumsA2[:], axis=AX.X)

                cc_in = dram.tile([1, 2], f32)
                cc_out = dram.tile([1, 2], f32)
                nc.sync.dma_start(out=cc_in[0:1, 0:1], in_=totA[0:1, 0:1])
                nc.sync.dma_start(out=cc_in[0:1, 1:2], in_=totA2[0:1, 0:1])
                nc.gpsimd.collective_compute(
                    "AllReduce", mybir.AluOpType.add,
                    replica_groups=[list(range(NCORES))],
                    ins=[cc_in.opt()], outs=[cc_out.opt()])
                g = stats.tile([128, 2], f32, tag="small2")
                nc.gpsimd.dma_start(out=g[:], in_=cc_out.to_broadcast((128, 2)))

                inv_n = 1.0 / float(B * T)
                mu = stats.tile([128, 1], f32, tag="small")
                nc.vector.tensor_scalar_mul(mu[:], g[:, 0:1], inv_n)
                ex2 = stats.tile([128, 1], f32, tag="small")
                nc.vector.tensor_scalar_mul(ex2[:], g[:, 1:2], inv_n)
                m2 = stats.tile([128, 1], f32, tag="small")
                nc.vector.tensor_mul(m2[:], mu[:], mu[:])
                var = stats.tile([128, 1], f32, tag="small")
                nc.vector.tensor_sub(var[:], ex2[:], m2[:])
                vep = stats.tile([128, 1], f32, tag="small")
                nc.vector.tensor_scalar_add(vep[:], var[:], BN_EPS)
                sd = stats.tile([128, 1], f32, tag="small")
                nc.scalar.sqrt(sd[:], vep[:])
                rstd = stats.tile([128, 1], f32, tag="small")
                nc.vector.reciprocal(rstd[:], sd[:])
                scl = singles.tile([128, 1], f32)
                nc.vector.tensor_mul(scl[:], rstd[:], gamma_sb[:])
                msc = stats.tile([128, 1], f32, tag="small")
                nc.vector.tensor_mul(msc[:], mu[:], scl[:])
                bias = singles.tile([128, 1], f32)
                nc.vector.tensor_sub(bias[:], beta_sb[:], msc[:])

                for b in range(BSH):
                    z_scr = scrp.tile([128, T], f32, tag="scr")
                    nc.scalar.activation(z_scr[:], a_all[:, b, :], AF.Tanh,
                                         bias=bias[:, 0:1], scale=scl[:, 0:1])
                    nc.scalar.activation(a_all[:, b, :], z_scr[:], AF.Exp,
                                         accum_out=Zb[:, b:b + 1])
                    nc.vector.reciprocal(rZ[:, b:b + 1], Zb[:, b:b + 1])
                    nc.vector.tensor_scalar_mul(a_all[:, b, :], a_all[:, b, :],
                                                rZ[:, b:b + 1])

                # ---- pass 2: weighted mean / sqmean over T ----
                for b in range(BSH):
                    meanT = stats.tile([128, KC], f32, tag="mean")
                    sqT = stats.tile([128, KC], f32, tag="sq")
                    for kc in range(KC):
                        x_t = xin.tile([128, T], f32)
                        nc.sync.dma_start(out=x_t[:],
                                          in_=x[b, kc * 128:(kc + 1) * 128, :])
                        y = ypool.tile([128, T], f32)
                        nc.vector.tensor_mul(y[:], x_t[:], a_all[:, b, :])
                        nc.scalar.activation(dump[:], y[:], AF.Copy,
                                             accum_out=meanT[:, kc:kc + 1])
                        z2 = zpool.tile([128, T], f32)
                        nc.vector.tensor_mul(z2[:], y[:], x_t[:])
                        nc.scalar.activation(dump[:], z2[:], AF.Copy,
                                             accum_out=sqT[:, kc:kc + 1])
                    m2o = stats.tile([128, KC], f32, tag="m2o")
                    nc.vector.tensor_mul(m2o[:], meanT[:], meanT[:])
                    vo = stats.tile([128, KC], f32, tag="vo")
                    nc.vector.tensor_sub(vo[:], sqT[:], m2o[:])
                    nc.vector.tensor_scalar_max(vo[:], vo[:], 1e-10)
                    stdT = stats.tile([128, KC], f32, tag="std")
                    nc.scalar.sqrt(stdT[:], vo[:])
                    nc.sync.dma_start(
                        out=out[b:b + 1, 0:C].rearrange("a (kc p) -> p (a kc)",
                                                        p=128),
                        in_=meanT[:])
                    nc.sync.dma_start(
                        out=out[b:b + 1, C:2 * C].rearrange("a (kc p) -> p (a kc)",
                                                            p=128),
                        in_=stdT[:])
    nc.compile()
    return nc


def _get_nc(nrep=1):
    if nrep not in _CACHE:
        _CACHE[nrep] = _build(nrep)
    return _CACHE[nrep]


def kernel(x, conv_w, conv_b, bn_gamma, bn_beta):
    from concourse.bass_utils import run_bass_kernel_spmd

    x = np.ascontiguousarray(np.asarray(x, dtype=np.float32))
    conv_w = np.asarray(conv_w, dtype=np.float32)
    bn_gamma = np.asarray(bn_gamma, dtype=np.float32)
    bn_beta = np.asarray(bn_beta, dtype=np.float32)

    nc = _get_nc()
    in_maps = [
        {"x": x[i * BSH:(i + 1) * BSH], "conv_w": conv_w,
         "bn_gamma": bn_gamma, "bn_beta": bn_beta}
        for i in range(NCORES)
    ]
    res = run_bass_kernel_spmd(nc, in_maps, core_ids=list(range(NCORES)))
    return np.concatenate([r["out"] for r in res.results], axis=0)


# revision 5
# speedup vs baseline: 10.1998x; 1.2237x over previous
"""AttentiveStatisticsPooling Trainium2 kernel (8 NeuronCores, batch-sharded).

Reference computation (B=32, C=1536, T=2000):
    a    = einsum('bct,c->bt', x, w) + cb          # 1x1 conv -> [B,T]
    a    = BN(a)  (batch stats over all B*T, biased var)    # syncBN via AllReduce
    attn = softmax(tanh(a), axis=T)
    mean = einsum('bct,bt->bc', x, attn)
    std  = sqrt(clip(E_attn[x^2] - mean^2, 1e-10))
    out  = concat([mean, std], axis=1)             # [B, 2C]

Sharding: batch across 8 cores (4 samples each). BN batch stats are exact:
local (sum a, sum a^2) are AllReduced across cores.  conv_b cancels out of
BN(a) exactly (shift invariance), so it is unused.

V2 dataflow per core:
  pass 1: stream x[b] tiles [128c x 2000t]; DVE rounds to f32r; PE matmul
          (f32r, 1 cyc/row) with conv_w replicated across 128 output
          partitions -> a broadcast to all partitions; stats on DVE/ACT.
  mid:    AllReduce 2 scalars; BN affine folded into ACT Tanh; ACT Exp with
          accum -> softmax; attn^T obtained via a DRAM round-trip.
  pass 2: hybrid.
    PE path (kc groups of 4): PE-transpose x blocks into PSUM [t, c];
          DVE/ACT copy PSUM->SBUF as f32r (ACT applies Square for the x^2
          path); PE matmul with attn^T as stationary -> per-(b, 512ch)
          row [1,512] of weighted sums, accumulated over T in PSUM.
    DVE path (remaining kc): DVE tensor_mul y=x*attn, z=y*x; ACT Copy
          activation with accum_out reduces over T.
"""

import numpy as np

B, C, T = 32, 1536, 2000
NCORES = 8
BSH = B // NCORES          # 4 samples per core
KC = C // 128              # 12 channel chunks
NTC = 4                    # pass-1 psum T chunks
TC = T // NTC              # 500 (<=512 psum bank)
NTT = 16                   # pass-2 T chunks of 128 (last is 80)
BN_EPS = 1e-5

# pass-2 split: kc groups of 4 handled on the PE path; rest on DVE path
PE_KCG = (0, 1)            # kc 0..7 on PE
DVE_KC = tuple(range(len(PE_KCG) * 4, KC))   # kc 8..11 on DVE
# fraction of PE-path plain copies shifted to ACT: tc % ACT_COPY_MOD == 0
ACT_COPY_MOD = 3

_CACHE = {}


def _build(nrep=1):
    import concourse.bacc as bacc
    import concourse.tile as tile
    import concourse.mybir as mybir
    from concourse.masks import make_identity

    f32 = mybir.dt.float32
    f32r = mybir.dt.float32r
    AF = mybir.ActivationFunctionType
    AX = mybir.AxisListType

    nc = bacc.Bacc("TRN2", target_bir_lowering=False, debug=False,
                   enable_asserts=True, num_devices=NCORES)
    x = nc.dram_tensor("x", [BSH, C, T], f32, kind="ExternalInput").ap()
    w = nc.dram_tensor("conv_w", [C], f32, kind="ExternalInput").ap()
    gamma = nc.dram_tensor("bn_gamma", [1], f32, kind="ExternalInput").ap()
    beta = nc.dram_tensor("bn_beta", [1], f32, kind="ExternalInput").ap()
    out = nc.dram_tensor("out", [BSH, 2 * C], f32, kind="ExternalOutput").ap()

    with tile.TileContext(nc) as tc:
        with (
            tc.tile_pool(name="singles", bufs=1) as singles,
            tc.tile_pool(name="xin", bufs=4) as xin,
            tc.tile_pool(name="xr", bufs=3) as xrp,
            tc.tile_pool(name="xt", bufs=3) as xtp,
            tc.tile_pool(name="ypool", bufs=2) as ypool,
            tc.tile_pool(name="zpool", bufs=2) as zpool,
            tc.tile_pool(name="scr", bufs=2) as scrp,
            tc.tile_pool(name="stats", bufs=4) as stats,
            tc.tile_pool(name="stage", bufs=2) as stagep,
            tc.tile_pool(name="pa", bufs=1, space="PSUM") as pap,
            tc.tile_pool(name="ptr", bufs=2, space="PSUM") as ptrp,
            tc.tile_pool(name="pacc", bufs=1, space="PSUM") as paccp,
            tc.tile_pool(name="dram", bufs=2, space="DRAM") as dram,
        ):
            # ---- setup ----
            w_sb = singles.tile([128, KC], f32)
            nc.sync.dma_start(out=w_sb[:], in_=w.rearrange("(kc p) -> p kc", p=128))
            ones = singles.tile([128, 128], f32)
            nc.vector.memset(ones[:], 1.0)
            w_rep = singles.tile([128, KC, 128], f32r)
            for kc in range(KC):
                nc.scalar.mul(w_rep[:, kc, :], ones[:], w_sb[:, kc:kc + 1])
            ident = singles.tile([128, 128], f32)
            make_identity(nc, ident[:])

            gamma_sb = singles.tile([128, 1], f32)
            nc.gpsimd.dma_start(out=gamma_sb[:], in_=gamma.to_broadcast((128, 1)))
            beta_sb = singles.tile([128, 1], f32)
            nc.gpsimd.dma_start(out=beta_sb[:], in_=beta.to_broadcast((128, 1)))

            a_all = singles.tile([128, BSH, T], f32)
            attnT = singles.tile([128, BSH, NTT], f32r)
            sumsA = singles.tile([128, BSH], f32)
            sumsA2 = singles.tile([128, BSH], f32)
            Zb = singles.tile([128, BSH], f32)
            rZ = singles.tile([128, BSH], f32)
            dump = singles.tile([128, T], f32)

            for _rep in range(nrep):
                # ---- pass 1 ----
                for b in range(BSH):
                    pa = pap.tile([128, NTC, 512], f32)
                    for kc in range(KC):
                        x_t = xin.tile([128, T], f32)
                        nc.sync.dma_start(out=x_t[:],
                                          in_=x[b, kc * 128:(kc + 1) * 128, :])
                        x_r = xrp.tile([128, T], f32r)
                        nc.vector.tensor_copy(x_r[:], x_t[:])
                        for i in range(NTC):
                            nc.tensor.matmul(pa[:, i, 0:TC], w_rep[:, kc, :],
                                             x_r[:, i * TC:(i + 1) * TC],
                                             start=(kc == 0), stop=(kc == KC - 1))
                    a_b = a_all[:, b, :].rearrange("p (i n) -> p i n", i=NTC)
                    nc.scalar.copy(a_b, pa[:, :, 0:TC])
                    nc.vector.reduce_sum(sumsA[:, b:b + 1], a_all[:, b, :], axis=AX.X)
                    sq_scr = scrp.tile([128, T], f32, tag="scr")
                    nc.scalar.activation(sq_scr[:], a_all[:, b, :], AF.Square,
                                         accum_out=sumsA2[:, b:b + 1])

                # ---- mid: BN stats allreduce, attn = softmax(tanh(bn(a))) ----
                totA = stats.tile([128, 1], f32, tag="small")
                nc.vector.reduce_sum(totA[:], sumsA[:], axis=AX.X)
                totA2 = stats.tile([128, 1], f32, tag="small")
                nc.vector.reduce_sum(totA2[:], sumsA2[:], axis=AX.X)

                cc_in = dram.tile([1, 2], f32)
                cc_out = dram.tile([1, 2], f32)
                nc.sync.dma_start(out=cc_in[0:1, 0:1], in_=totA[0:1, 0:1])
                nc.sync.dma_start(out=cc_in[0:1, 1:2], in_=totA2[0:1, 0:1])
                nc.gpsimd.collective_compute(
                    "AllReduce", mybir.AluOpType.add,
                    replica_groups=[list(range(NCORES))],
                    ins=[cc_in.opt()], outs=[cc_out.opt()])
                g = stats.tile([128, 2], f32, tag="small2")
                nc.gpsimd.dma_start(out=g[:], in_=cc_out.to_broadcast((128, 2)))

                inv_n = 1.0 / float(B * T)
                mu = stats.tile([128, 1], f32, tag="small")
                nc.vector.tensor_scalar_mul(mu[:], g[:, 0:1], inv_n)
                ex2 = stats.tile([128, 1], f32, tag="small")
                nc.vector.tensor_scalar_mul(ex2[:], g[:, 1:2], inv_n)
                m2 = stats.tile([128, 1], f32, tag="small")
                nc.vector.tensor_mul(m2[:], mu[:], mu[:])
                var = stats.tile([128, 1], f32, tag="small")
                nc.vector.tensor_sub(var[:], ex2[:], m2[:])
                vep = stats.tile([128, 1], f32, tag="small")
                nc.vector.tensor_scalar_add(vep[:], var[:], BN_EPS)
                sd = stats.tile([128, 1], f32, tag="small")
                nc.scalar.sqrt(sd[:], vep[:])
                rstd = stats.tile([128, 1], f32, tag="small")
                nc.vector.reciprocal(rstd[:], sd[:])
                scl = singles.tile([128, 1], f32)
                nc.vector.tensor_mul(scl[:], rstd[:], gamma_sb[:])
                msc = stats.tile([128, 1], f32, tag="small")
                nc.vector.tensor_mul(msc[:], mu[:], scl[:])
                bias = singles.tile([128, 1], f32)
                nc.vector.tensor_sub(bias[:], beta_sb[:], msc[:])

                for b in range(BSH):
                    z_scr = scrp.tile([128, T], f32, tag="scr")
                    nc.scalar.activation(z_scr[:], a_all[:, b, :], AF.Tanh,
                                         bias=bias[:, 0:1], scale=scl[:, 0:1])
                    nc.scalar.activation(a_all[:, b, :], z_scr[:], AF.Exp,
                                         accum_out=Zb[:, b:b + 1])
                    nc.vector.reciprocal(rZ[:, b:b + 1], Zb[:, b:b + 1])
                    nc.vector.tensor_scalar_mul(a_all[:, b, :], a_all[:, b, :],
                                                rZ[:, b:b + 1])
                    # attn^T via DRAM round trip: [1, 2000] -> [128, 15] + [80, 1]
                    abounce = dram.tile([1, T], f32)
                    nc.sync.dma_start(out=abounce[:], in_=a_all[0:1, b, :])
                    at_f = stats.tile([128, NTT], f32, tag="atf")
                    nc.vector.memset(at_f[:], 0.0)
                    nc.sync.dma_start(
                        out=at_f[:, 0:NTT - 1],
                        in_=abounce[0, 0:(NTT - 1) * 128].rearrange(
                            "(tc p) -> p tc", p=128))
                    nc.sync.dma_start(
                        out=at_f[0:T - (NTT - 1) * 128, NTT - 1:NTT],
                        in_=abounce[0, (NTT - 1) * 128:T].rearrange(
                            "(p one) -> p one", one=1))
                    nc.vector.tensor_copy(attnT[:, b, :], at_f[:])

                # ---- pass 2 ----
                for b in range(BSH):
                    # PE path: kc groups of 4 -> channels [kcg*512, kcg*512+512)
                    for kcg in PE_KCG:
                        xts = []
                        for j in range(4):
                            kc = kcg * 4 + j
                            x_t = xin.tile([128, T], f32)
                            nc.sync.dma_start(
                                out=x_t[:], in_=x[b, kc * 128:(kc + 1) * 128, :])
                            xts.append(x_t)
                        acc = paccp.tile([1, 2, 512], f32)
                        for t in range(NTT):
                            tw = min(128, T - t * 128)
                            ptr = ptrp.tile([128, 4, 128], f32)
                            for j in range(4):
                                nc.tensor.transpose(
                                    ptr[0:tw, j, :],
                                    xts[j][:, t * 128:t * 128 + tw],
                                    ident[:])
                            xT = xtp.tile([128, 4, 128], f32r, tag="xT")
                            if t % ACT_COPY_MOD == 0:
                                nc.scalar.copy(xT[:], ptr[:])
                            else:
                                nc.vector.tensor_copy(xT[:], ptr[:])
                            x2T = xtp.tile([128, 4, 128], f32r, tag="x2T")
                            nc.scalar.activation(x2T[:], ptr[:], AF.Square)
                            nc.tensor.matmul(
                                acc[:, 0, :], attnT[:, b, t:t + 1],
                                xT[:].rearrange("p a c -> p (a c)"),
                                start=(t == 0), stop=(t == NTT - 1))
                            nc.tensor.matmul(
                                acc[:, 1, :], attnT[:, b, t:t + 1],
                                x2T[:].rearrange("p a c -> p (a c)"),
                                start=(t == 0), stop=(t == NTT - 1))
                        stg = stagep.tile([1, 2, 512], f32)
                        nc.scalar.copy(stg[:], acc[:])
                        m2r = stagep.tile([1, 512], f32, tag="m2r")
                        nc.vector.tensor_mul(m2r[:], stg[:, 0, :], stg[:, 0, :])
                        vor = stagep.tile([1, 512], f32, tag="vor")
                        nc.vector.tensor_sub(vor[:], stg[:, 1, :], m2r[:])
                        nc.vector.tensor_scalar_max(vor[:], vor[:], 1e-10)
                        stdr = stagep.tile([1, 512], f32, tag="stdr")
                        nc.scalar.sqrt(stdr[:], vor[:])
                        nc.sync.dma_start(
                            out=out[b:b + 1, kcg * 512:(kcg + 1) * 512],
                            in_=stg[:, 0, :])
                        nc.sync.dma_start(
                            out=out[b:b + 1, C + kcg * 512:C + (kcg + 1) * 512],
                            in_=stdr[:])

                    # DVE path: remaining kc
                    nd = len(DVE_KC)
                    meanT = stats.tile([128, nd], f32, tag="mean")
                    sqT = stats.tile([128, nd], f32, tag="sq")
                    for ci, kc in enumerate(DVE_KC):
                        x_t = xin.tile([128, T], f32)
                        nc.sync.dma_start(out=x_t[:],
                                          in_=x[b, kc * 128:(kc + 1) * 128, :])
                        y = ypool.tile([128, T], f32)
                        nc.vector.tensor_mul(y[:], x_t[:], a_all[:, b, :])
                        nc.scalar.activation(dump[:], y[:], AF.Copy,
                                             accum_out=meanT[:, ci:ci + 1])
                        z2 = zpool.tile([128, T], f32)
                        nc.vector.tensor_mul(z2[:], y[:], x_t[:])
                        nc.scalar.activation(dump[:], z2[:], AF.Copy,
                                             accum_out=sqT[:, ci:ci + 1])
                    m2o = stats.tile([128, nd], f32, tag="m2o")
                    nc.vector.tensor_mul(m2o[:], meanT[:], meanT[:])
                    vo = stats.tile([128, nd], f32, tag="vo")
                    nc.vector.tensor_sub(vo[:], sqT[:], m2o[:])
                    nc.vector.tensor_scalar_max(vo[:], vo[:], 1e-10)
                    stdT = stats.tile([128, nd], f32, tag="std")
                    nc.scalar.sqrt(stdT[:], vo[:])
                    c0 = DVE_KC[0] * 128
                    c1 = (DVE_KC[-1] + 1) * 128
                    nc.sync.dma_start(
                        out=out[b:b + 1, c0:c1].rearrange(
                            "a (kc p) -> p (a kc)", p=128),
                        in_=meanT[:])
                    nc.sync.dma_start(
                        out=out[b:b + 1, C + c0:C + c1].rearrange(
                            "a (kc p) -> p (a kc)", p=128),
                        in_=stdT[:])
    nc.compile()
    return nc


def _get_nc(nrep=1):
    if nrep not in _CACHE:
        _CACHE[nrep] = _build(nrep)
    return _CACHE[nrep]


def kernel(x, conv_w, conv_b, bn_gamma, bn_beta):
    from concourse.bass_utils import run_bass_kernel_spmd

    x = np.ascontiguousarray(np.asarray(x, dtype=np.float32))
    conv_w = np.asarray(conv_w, dtype=np.float32)
    bn_gamma = np.asarray(bn_gamma, dtype=np.float32)
    bn_beta = np.asarray(bn_beta, dtype=np.float32)

    nc = _get_nc()
    in_maps = [
        {"x": x[i * BSH:(i + 1) * BSH], "conv_w": conv_w,
         "bn_gamma": bn_gamma, "bn_beta": bn_beta}
        for i in range(NCORES)
    ]
    res = run_bass_kernel_spmd(nc, in_maps, core_ids=list(range(NCORES)))
    return np.concatenate([r["out"] for r in res.results], axis=0)


# revision 9
# speedup vs baseline: 10.4101x; 1.0206x over previous
"""AttentiveStatisticsPooling Trainium2 kernel (8 NeuronCores, batch-sharded).

Reference computation (B=32, C=1536, T=2000):
    a    = einsum('bct,c->bt', x, w) + cb          # 1x1 conv -> [B,T]
    a    = BN(a)  (batch stats over all B*T, biased var)    # syncBN via AllReduce
    attn = softmax(tanh(a), axis=T)
    mean = einsum('bct,bt->bc', x, attn)
    std  = sqrt(clip(E_attn[x^2] - mean^2, 1e-10))
    out  = concat([mean, std], axis=1)             # [B, 2C]

Sharding: batch across 8 cores (4 samples each). BN batch stats are exact:
local (sum a, sum a^2) are AllReduced across cores.  conv_b cancels out of
BN(a) exactly (shift invariance), so it is unused.

V2 dataflow per core:
  pass 1: stream x[b] tiles [128c x 2000t]; DVE rounds to f32r; PE matmul
          (f32r, 1 cyc/row) with conv_w replicated across 128 output
          partitions -> a broadcast to all partitions; stats on DVE/ACT.
  mid:    AllReduce 2 scalars; BN affine folded into ACT Tanh; ACT Exp with
          accum -> softmax; attn^T obtained via a DRAM round-trip.
  pass 2: hybrid.
    PE path (kc groups of 4): PE-transpose x blocks into PSUM [t, c];
          DVE/ACT copy PSUM->SBUF as f32r (ACT applies Square for the x^2
          path); PE matmul with attn^T as stationary -> per-(b, 512ch)
          row [1,512] of weighted sums, accumulated over T in PSUM.
    DVE path (remaining kc): DVE tensor_mul y=x*attn, z=y*x; ACT Copy
          activation with accum_out reduces over T.
"""

import numpy as np

B, C, T = 32, 1536, 2000
NCORES = 8
BSH = B // NCORES          # 4 samples per core
KC = C // 128              # 12 channel chunks
NTC = 4                    # pass-1 psum T chunks
TC = T // NTC              # 500 (<=512 psum bank)
NTT = 16                   # pass-2 T chunks of 128 (last is 80)
BN_EPS = 1e-5

# pass-2 split: kc groups of 4 handled on the PE path; rest on DVE path
import os as _os
_n_pe_groups = int(_os.environ.get("ASP_PE_GROUPS", "2"))
PE_KCG = tuple(range(_n_pe_groups))
DVE_KC = tuple(range(len(PE_KCG) * 4, KC))
# fraction of PE-path plain copies shifted to ACT: tc % ACT_COPY_MOD == 0
ACT_COPY_MOD = int(_os.environ.get("ASP_ACT_COPY_MOD", "3"))

_CACHE = {}


def _build(nrep=1):
    import os
    PHASE = os.environ.get("ASP_PHASE", "all")
    import concourse.bacc as bacc
    import concourse.tile as tile
    import concourse.mybir as mybir
    from concourse.masks import make_identity

    f32 = mybir.dt.float32
    f32r = mybir.dt.float32r
    AF = mybir.ActivationFunctionType
    AX = mybir.AxisListType

    nc = bacc.Bacc("TRN2", target_bir_lowering=False, debug=False,
                   enable_asserts=True, num_devices=NCORES)
    x = nc.dram_tensor("x", [BSH, C, T], f32, kind="ExternalInput").ap()
    w = nc.dram_tensor("conv_w", [C], f32, kind="ExternalInput").ap()
    gamma = nc.dram_tensor("bn_gamma", [1], f32, kind="ExternalInput").ap()
    beta = nc.dram_tensor("bn_beta", [1], f32, kind="ExternalInput").ap()
    out = nc.dram_tensor("out", [BSH, 2 * C], f32, kind="ExternalOutput").ap()

    with tile.TileContext(nc) as tc:
        with (
            tc.tile_pool(name="singles", bufs=1) as singles,
            tc.tile_pool(name="xin", bufs=4) as xin,
            tc.tile_pool(name="xr", bufs=3) as xrp,
            tc.tile_pool(name="xt", bufs=3) as xtp,
            tc.tile_pool(name="ypool", bufs=2) as ypool,
            tc.tile_pool(name="zpool", bufs=2) as zpool,
            tc.tile_pool(name="scr", bufs=2) as scrp,
            tc.tile_pool(name="stats", bufs=4) as stats,
            tc.tile_pool(name="stage", bufs=2) as stagep,
            tc.tile_pool(name="pa", bufs=1, space="PSUM") as pap,
            tc.tile_pool(name="ptr", bufs=2, space="PSUM") as ptrp,
            tc.tile_pool(name="pacc", bufs=1, space="PSUM") as paccp,
            tc.tile_pool(name="dram", bufs=2, space="DRAM") as dram,
        ):
            # ---- setup ----
            w_sb = singles.tile([128, KC], f32)
            nc.sync.dma_start(out=w_sb[:], in_=w.rearrange("(kc p) -> p kc", p=128))
            ones = singles.tile([128, 128], f32)
            nc.vector.memset(ones[:], 1.0)
            w_rep = singles.tile([128, KC, 128], f32r)
            for kc in range(KC):
                nc.scalar.mul(w_rep[:, kc, :], ones[:], w_sb[:, kc:kc + 1])
            ident = singles.tile([128, 128], f32)
            make_identity(nc, ident[:])
            ones_r = singles.tile([128, 1], f32r)
            nc.vector.tensor_copy(ones_r[:], ones[:, 0:1])

            gamma_sb = singles.tile([128, 1], f32)
            nc.gpsimd.dma_start(out=gamma_sb[:], in_=gamma.to_broadcast((128, 1)))
            beta_sb = singles.tile([128, 1], f32)
            nc.gpsimd.dma_start(out=beta_sb[:], in_=beta.to_broadcast((128, 1)))

            a_all = singles.tile([128, BSH, T], f32)
            attnT = singles.tile([128, BSH, NTT], f32)
            sqattnT = singles.tile([128, BSH, NTT], f32)
            sumsA = singles.tile([128, BSH], f32)
            sumsA2 = singles.tile([128, BSH], f32)
            Zb = singles.tile([128, BSH], f32)
            rZ = singles.tile([128, BSH], f32)
            dump = singles.tile([128, T], f32)

            for _rep in range(nrep):
                # ---- pass 1 ----
                if PHASE == "p2":
                    for b in range(BSH):
                        nc.vector.memset(a_all[:, b, :], 0.0005)
                    nc.vector.memset(sumsA[:], 1.0)
                    nc.vector.memset(sumsA2[:], 2.0)
                for b in range(BSH if PHASE != "p2" else 0):
                    pa = pap.tile([128, NTC, 512], f32)
                    for kc in range(KC):
                        x_t = xin.tile([128, T], f32)
                        nc.sync.dma_start(out=x_t[:],
                                          in_=x[b, kc * 128:(kc + 1) * 128, :])
                        x_r = xrp.tile([128, T], f32r)
                        nc.vector.tensor_copy(x_r[:], x_t[:])
                        for i in range(NTC):
                            nc.tensor.matmul(pa[:, i, 0:TC], w_rep[:, kc, :],
                                             x_r[:, i * TC:(i + 1) * TC],
                                             start=(kc == 0), stop=(kc == KC - 1))
                    a_b = a_all[:, b, :].rearrange("p (i n) -> p i n", i=NTC)
                    nc.scalar.copy(a_b, pa[:, :, 0:TC])
                    nc.vector.reduce_sum(sumsA[:, b:b + 1], a_all[:, b, :], axis=AX.X)
                    sq_scr = scrp.tile([128, T], f32, tag="scr")
                    nc.scalar.activation(sq_scr[:], a_all[:, b, :], AF.Square,
                                         accum_out=sumsA2[:, b:b + 1])

                # ---- mid: BN stats allreduce, attn = softmax(tanh(bn(a))) ----
                if PHASE == "p1only":
                    nc.sync.dma_start(out=out[0:1, 0:2].rearrange("a b -> b a"),
                                      in_=sumsA[0:2, 0:1])
                    continue
                totA = stats.tile([128, 1], f32, tag="small")
                nc.vector.reduce_sum(totA[:], sumsA[:], axis=AX.X)
                totA2 = stats.tile([128, 1], f32, tag="small")
                nc.vector.reduce_sum(totA2[:], sumsA2[:], axis=AX.X)

                cc_in = dram.tile([1, 2], f32)
                cc_out = dram.tile([1, 2], f32)
                nc.sync.dma_start(out=cc_in[0:1, 0:1], in_=totA[0:1, 0:1])
                nc.sync.dma_start(out=cc_in[0:1, 1:2], in_=totA2[0:1, 0:1])
                nc.gpsimd.collective_compute(
                    "AllReduce", mybir.AluOpType.add,
                    replica_groups=[list(range(NCORES))],
                    ins=[cc_in.opt()], outs=[cc_out.opt()])
                g = stats.tile([128, 2], f32, tag="small2")
                nc.gpsimd.dma_start(out=g[:], in_=cc_out.to_broadcast((128, 2)))

                inv_n = 1.0 / float(B * T)
                mu = stats.tile([128, 1], f32, tag="small")
                nc.vector.tensor_scalar_mul(mu[:], g[:, 0:1], inv_n)
                ex2 = stats.tile([128, 1], f32, tag="small")
                nc.vector.tensor_scalar_mul(ex2[:], g[:, 1:2], inv_n)
                m2 = stats.tile([128, 1], f32, tag="small")
                nc.vector.tensor_mul(m2[:], mu[:], mu[:])
                var = stats.tile([128, 1], f32, tag="small")
                nc.vector.tensor_sub(var[:], ex2[:], m2[:])
                vep = stats.tile([128, 1], f32, tag="small")
                nc.vector.tensor_scalar_add(vep[:], var[:], BN_EPS)
                sd = stats.tile([128, 1], f32, tag="small")
                nc.scalar.sqrt(sd[:], vep[:])
                rstd = stats.tile([128, 1], f32, tag="small")
                nc.vector.reciprocal(rstd[:], sd[:])
                scl = singles.tile([128, 1], f32)
                nc.vector.tensor_mul(scl[:], rstd[:], gamma_sb[:])
                msc = stats.tile([128, 1], f32, tag="small")
                nc.vector.tensor_mul(msc[:], mu[:], scl[:])
                bias = singles.tile([128, 1], f32)
                nc.vector.tensor_sub(bias[:], beta_sb[:], msc[:])

                for b in range(BSH):
                    z_scr = scrp.tile([128, T], f32, tag="scr")
                    nc.scalar.activation(z_scr[:], a_all[:, b, :], AF.Tanh,
                                         bias=bias[:, 0:1], scale=scl[:, 0:1])
                    nc.scalar.activation(a_all[:, b, :], z_scr[:], AF.Exp,
                                         accum_out=Zb[:, b:b + 1])
                    nc.vector.reciprocal(rZ[:, b:b + 1], Zb[:, b:b + 1])
                    nc.vector.tensor_scalar_mul(a_all[:, b, :], a_all[:, b, :],
                                                rZ[:, b:b + 1])
                    # attn^T via DRAM round trip: [1, 2000] -> [128, 15] + [80, 1]
                    abounce = dram.tile([1, T], f32)
                    nc.sync.dma_start(out=abounce[:], in_=a_all[0:1, b, :])
                    nc.vector.memset(attnT[:, b, :], 0.0)
                    nc.sync.dma_start(
                        out=attnT[:, b, 0:NTT - 1],
                        in_=abounce[0, 0:(NTT - 1) * 128].rearrange(
                            "(tc p) -> p tc", p=128))
                    nc.sync.dma_start(
                        out=attnT[0:T - (NTT - 1) * 128, b, NTT - 1:NTT],
                        in_=abounce[0, (NTT - 1) * 128:T].rearrange(
                            "(p one) -> p one", one=1))
                    nc.scalar.sqrt(sqattnT[:, b, :], attnT[:, b, :])

                # ---- pass 2 ----
                if PHASE == "p1":
                    nc.sync.dma_start(out=out[0:1, 0:2].rearrange("a b -> b a"),
                                      in_=rZ[0:2, 0:1])
                    continue
                for b in range(BSH):
                    # PE path: kc groups of 4 -> channels [kcg*512, kcg*512+512)
                    for kcg in PE_KCG:
                        xts = []
                        for j in range(4):
                            kc = kcg * 4 + j
                            x_t = xin.tile([128, T], f32)
                            nc.sync.dma_start(
                                out=x_t[:], in_=x[b, kc * 128:(kc + 1) * 128, :])
                            xts.append(x_t)
                        acc = paccp.tile([1, 2, 512], f32)
                        for t in range(NTT):
                            tw = min(128, T - t * 128)
                            ptr = ptrp.tile([128, 4, 128], f32)
                            for j in range(4):
                                nc.tensor.transpose(
                                    ptr[0:tw, j, :],
                                    xts[j][:, t * 128:t * 128 + tw],
                                    ident[:])
                            y4 = xtp.tile([128, 4, 128], f32r, tag="xT")
                            nc.vector.tensor_scalar_mul(
                                y4[:], ptr[:], attnT[:, b, t:t + 1])
                            z4 = xtp.tile([128, 4, 128], f32r, tag="x2T")
                            nc.scalar.activation(z4[:], ptr[:], AF.Square,
                                                 scale=sqattnT[:, b, t:t + 1])
                            nc.tensor.matmul(
                                acc[:, 0, :], ones_r[:],
                                y4[:].rearrange("p a c -> p (a c)"),
                                start=(t == 0), stop=(t == NTT - 1))
                            nc.tensor.matmul(
                                acc[:, 1, :], ones_r[:],
                                z4[:].rearrange("p a c -> p (a c)"),
                                start=(t == 0), stop=(t == NTT - 1))
                        stg = stagep.tile([1, 2, 512], f32)
                        nc.scalar.copy(stg[:], acc[:])
                        m2r = stagep.tile([1, 512], f32, tag="m2r")
                        nc.vector.tensor_mul(m2r[:], stg[:, 0, :], stg[:, 0, :])
                        vor = stagep.tile([1, 512], f32, tag="vor")
                        nc.vector.tensor_sub(vor[:], stg[:, 1, :], m2r[:])
                        nc.vector.tensor_scalar_max(vor[:], vor[:], 1e-10)
                        stdr = stagep.tile([1, 512], f32, tag="stdr")
                        nc.scalar.sqrt(stdr[:], vor[:])
                        nc.sync.dma_start(
                            out=out[b:b + 1, kcg * 512:(kcg + 1) * 512],
                            in_=stg[:, 0, :])
                        nc.sync.dma_start(
                            out=out[b:b + 1, C + kcg * 512:C + (kcg + 1) * 512],
                            in_=stdr[:])

                    # DVE path: remaining kc
                    if not DVE_KC:
                        continue
                    nd = len(DVE_KC)
                    meanT = stats.tile([128, nd], f32, tag="mean")
                    sqT = stats.tile([128, nd], f32, tag="sq")
                    for ci, kc in enumerate(DVE_KC):
                        x_t = xin.tile([128, T], f32)
                        nc.sync.dma_start(out=x_t[:],
                                          in_=x[b, kc * 128:(kc + 1) * 128, :])
                        y = ypool.tile([128, T], f32)
                        nc.vector.tensor_mul(y[:], x_t[:], a_all[:, b, :])
                        nc.scalar.activation(dump[:], y[:], AF.Copy,
                                             accum_out=meanT[:, ci:ci + 1])
                        z2 = zpool.tile([128, T], f32)
                        nc.vector.tensor_mul(z2[:], y[:], x_t[:])
                        nc.scalar.activation(dump[:], z2[:], AF.Copy,
                                             accum_out=sqT[:, ci:ci + 1])
                    m2o = stats.tile([128, nd], f32, tag="m2o")
                    nc.vector.tensor_mul(m2o[:], meanT[:], meanT[:])
                    vo = stats.tile([128, nd], f32, tag="vo")
                    nc.vector.tensor_sub(vo[:], sqT[:], m2o[:])
                    nc.vector.tensor_scalar_max(vo[:], vo[:], 1e-10)
                    stdT = stats.tile([128, nd], f32, tag="std")
                    nc.scalar.sqrt(stdT[:], vo[:])
                    c0 = DVE_KC[0] * 128
                    c1 = (DVE_KC[-1] + 1) * 128
                    nc.sync.dma_start(
                        out=out[b:b + 1, c0:c1].rearrange(
                            "a (kc p) -> p (a kc)", p=128),
                        in_=meanT[:])
                    nc.sync.dma_start(
                        out=out[b:b + 1, C + c0:C + c1].rearrange(
                            "a (kc p) -> p (a kc)", p=128),
                        in_=stdT[:])
    nc.compile()
    return nc


def _get_nc(nrep=1):
    if nrep not in _CACHE:
        _CACHE[nrep] = _build(nrep)
    return _CACHE[nrep]


def kernel(x, conv_w, conv_b, bn_gamma, bn_beta):
    from concourse.bass_utils import run_bass_kernel_spmd

    x = np.ascontiguousarray(np.asarray(x, dtype=np.float32))
    conv_w = np.asarray(conv_w, dtype=np.float32)
    bn_gamma = np.asarray(bn_gamma, dtype=np.float32)
    bn_beta = np.asarray(bn_beta, dtype=np.float32)

    nc = _get_nc()
    in_maps = [
        {"x": x[i * BSH:(i + 1) * BSH], "conv_w": conv_w,
         "bn_gamma": bn_gamma, "bn_beta": bn_beta}
        for i in range(NCORES)
    ]
    res = run_bass_kernel_spmd(nc, in_maps, core_ids=list(range(NCORES)))
    return np.concatenate([r["out"] for r in res.results], axis=0)


# revision 10
# speedup vs baseline: 10.4129x; 1.0003x over previous
"""AttentiveStatisticsPooling Trainium2 kernel (8 NeuronCores, batch-sharded).

Reference computation (B=32, C=1536, T=2000):
    a    = einsum('bct,c->bt', x, w) + cb          # 1x1 conv -> [B,T]
    a    = BN(a)  (batch stats over all B*T, biased var)    # syncBN via AllReduce
    attn = softmax(tanh(a), axis=T)
    mean = einsum('bct,bt->bc', x, attn)
    std  = sqrt(clip(E_attn[x^2] - mean^2, 1e-10))
    out  = concat([mean, std], axis=1)             # [B, 2C]

Sharding: batch across 8 cores (4 samples each). BN batch stats are exact:
local (sum a, sum a^2) are AllReduced across cores.  conv_b cancels out of
BN(a) exactly (shift invariance), so it is unused.

V2 dataflow per core:
  pass 1: stream x[b] tiles [128c x 2000t]; DVE rounds to f32r; PE matmul
          (f32r, 1 cyc/row) with conv_w replicated across 128 output
          partitions -> a broadcast to all partitions; stats on DVE/ACT.
  mid:    AllReduce 2 scalars; BN affine folded into ACT Tanh; ACT Exp with
          accum -> softmax; attn^T obtained via a DRAM round-trip.
  pass 2: hybrid.
    PE path (kc groups of 4): PE-transpose x blocks into PSUM [t, c];
          DVE/ACT copy PSUM->SBUF as f32r (ACT applies Square for the x^2
          path); PE matmul with attn^T as stationary -> per-(b, 512ch)
          row [1,512] of weighted sums, accumulated over T in PSUM.
    DVE path (remaining kc): DVE tensor_mul y=x*attn, z=y*x; ACT Copy
          activation with accum_out reduces over T.
"""

import numpy as np

B, C, T = 32, 1536, 2000
NCORES = 8
BSH = B // NCORES          # 4 samples per core
KC = C // 128              # 12 channel chunks
NTC = 4                    # pass-1 psum T chunks
TC = T // NTC              # 500 (<=512 psum bank)
NTT = 16                   # pass-2 T chunks of 128 (last is 80)
BN_EPS = 1e-5

import os as _os

_CACHE = {}


def _build(nrep=1, phase=None, pe_groups=None):
    PHASE = phase if phase is not None else _os.environ.get("ASP_PHASE", "all")
    if pe_groups is None:
        pe_groups = int(_os.environ.get("ASP_PE_GROUPS", "2"))
    PE_KCG = tuple(range(pe_groups))
    DVE_KC = tuple(range(len(PE_KCG) * 4, KC))
    import concourse.bacc as bacc
    import concourse.tile as tile
    import concourse.mybir as mybir
    from concourse.masks import make_identity

    f32 = mybir.dt.float32
    f32r = mybir.dt.float32r
    AF = mybir.ActivationFunctionType
    AX = mybir.AxisListType

    nc = bacc.Bacc("TRN2", target_bir_lowering=False, debug=False,
                   enable_asserts=True, num_devices=NCORES)
    x = nc.dram_tensor("x", [BSH, C, T], f32, kind="ExternalInput").ap()
    w = nc.dram_tensor("conv_w", [C], f32, kind="ExternalInput").ap()
    gamma = nc.dram_tensor("bn_gamma", [1], f32, kind="ExternalInput").ap()
    beta = nc.dram_tensor("bn_beta", [1], f32, kind="ExternalInput").ap()
    out = nc.dram_tensor("out", [BSH, 2 * C], f32, kind="ExternalOutput").ap()

    with tile.TileContext(nc) as tc:
        with (
            tc.tile_pool(name="singles", bufs=1) as singles,
            tc.tile_pool(name="xin", bufs=4) as xin,
            tc.tile_pool(name="xr", bufs=3) as xrp,
            tc.tile_pool(name="xt", bufs=3) as xtp,
            tc.tile_pool(name="ypool", bufs=2) as ypool,
            tc.tile_pool(name="zpool", bufs=2) as zpool,
            tc.tile_pool(name="scr", bufs=2) as scrp,
            tc.tile_pool(name="stats", bufs=4) as stats,
            tc.tile_pool(name="stage", bufs=2) as stagep,
            tc.tile_pool(name="pa", bufs=1, space="PSUM") as pap,
            tc.tile_pool(name="ptr", bufs=2, space="PSUM") as ptrp,
            tc.tile_pool(name="pacc", bufs=1, space="PSUM") as paccp,
            tc.tile_pool(name="dram", bufs=2, space="DRAM") as dram,
        ):
            # ---- setup ----
            w_sb = singles.tile([128, KC], f32)
            nc.sync.dma_start(out=w_sb[:], in_=w.rearrange("(kc p) -> p kc", p=128))
            ones = singles.tile([128, 128], f32)
            nc.vector.memset(ones[:], 1.0)
            w_rep = singles.tile([128, KC, 128], f32r)
            for kc in range(KC):
                nc.scalar.mul(w_rep[:, kc, :], ones[:], w_sb[:, kc:kc + 1])
            ident = singles.tile([128, 128], f32)
            make_identity(nc, ident[:])
            ones_r = singles.tile([128, 1], f32r)
            nc.vector.tensor_copy(ones_r[:], ones[:, 0:1])

            gamma_sb = singles.tile([128, 1], f32)
            nc.gpsimd.dma_start(out=gamma_sb[:], in_=gamma.to_broadcast((128, 1)))
            beta_sb = singles.tile([128, 1], f32)
            nc.gpsimd.dma_start(out=beta_sb[:], in_=beta.to_broadcast((128, 1)))

            a_all = singles.tile([128, BSH, T], f32)
            attnT = singles.tile([128, BSH, NTT], f32)
            sqattnT = singles.tile([128, BSH, NTT], f32)
            sumsA = singles.tile([128, BSH], f32)
            sumsA2 = singles.tile([128, BSH], f32)
            Zb = singles.tile([128, BSH], f32)
            rZ = singles.tile([128, BSH], f32)
            dump = singles.tile([128, T], f32)

            for _rep in range(nrep):
                # ---- pass 1 ----
                if PHASE == "p2":
                    for b in range(BSH):
                        nc.vector.memset(a_all[:, b, :], 0.0005)
                    nc.vector.memset(sumsA[:], 1.0)
                    nc.vector.memset(sumsA2[:], 2.0)
                for b in range(BSH if PHASE != "p2" else 0):
                    pa = pap.tile([128, NTC, 512], f32)
                    for kc in range(KC):
                        x_t = xin.tile([128, T], f32)
                        nc.sync.dma_start(out=x_t[:],
                                          in_=x[b, kc * 128:(kc + 1) * 128, :])
                        x_r = xrp.tile([128, T], f32r)
                        nc.vector.tensor_copy(x_r[:], x_t[:])
                        for i in range(NTC):
                            nc.tensor.matmul(pa[:, i, 0:TC], w_rep[:, kc, :],
                                             x_r[:, i * TC:(i + 1) * TC],
                                             start=(kc == 0), stop=(kc == KC - 1))
                    a_b = a_all[:, b, :].rearrange("p (i n) -> p i n", i=NTC)
                    nc.scalar.copy(a_b, pa[:, :, 0:TC])
                    nc.vector.reduce_sum(sumsA[:, b:b + 1], a_all[:, b, :], axis=AX.X)
                    sq_scr = scrp.tile([128, T], f32, tag="scr")
                    nc.scalar.activation(sq_scr[:], a_all[:, b, :], AF.Square,
                                         accum_out=sumsA2[:, b:b + 1])

                # ---- mid: BN stats allreduce, attn = softmax(tanh(bn(a))) ----
                if PHASE == "p1only":
                    nc.sync.dma_start(out=out[0:1, 0:2].rearrange("a b -> b a"),
                                      in_=sumsA[0:2, 0:1])
                    continue
                totA = stats.tile([128, 1], f32, tag="small")
                nc.vector.reduce_sum(totA[:], sumsA[:], axis=AX.X)
                totA2 = stats.tile([128, 1], f32, tag="small")
                nc.vector.reduce_sum(totA2[:], sumsA2[:], axis=AX.X)

                cc_in = dram.tile([1, 2], f32)
                cc_out = dram.tile([1, 2], f32)
                nc.sync.dma_start(out=cc_in[0:1, 0:1], in_=totA[0:1, 0:1])
                nc.sync.dma_start(out=cc_in[0:1, 1:2], in_=totA2[0:1, 0:1])
                nc.gpsimd.collective_compute(
                    "AllReduce", mybir.AluOpType.add,
                    replica_groups=[list(range(NCORES))],
                    ins=[cc_in.opt()], outs=[cc_out.opt()])
                g = stats.tile([128, 2], f32, tag="small2")
                nc.gpsimd.dma_start(out=g[:], in_=cc_out.to_broadcast((128, 2)))

                inv_n = 1.0 / float(B * T)
                mu = stats.tile([128, 1], f32, tag="small")
                nc.vector.tensor_scalar_mul(mu[:], g[:, 0:1], inv_n)
                ex2 = stats.tile([128, 1], f32, tag="small")
                nc.vector.tensor_scalar_mul(ex2[:], g[:, 1:2], inv_n)
                m2 = stats.tile([128, 1], f32, tag="small")
                nc.vector.tensor_mul(m2[:], mu[:], mu[:])
                var = stats.tile([128, 1], f32, tag="small")
                nc.vector.tensor_sub(var[:], ex2[:], m2[:])
                vep = stats.tile([128, 1], f32, tag="small")
                nc.vector.tensor_scalar_add(vep[:], var[:], BN_EPS)
                sd = stats.tile([128, 1], f32, tag="small")
                nc.scalar.sqrt(sd[:], vep[:])
                rstd = stats.tile([128, 1], f32, tag="small")
                nc.vector.reciprocal(rstd[:], sd[:])
                scl = singles.tile([128, 1], f32)
                nc.vector.tensor_mul(scl[:], rstd[:], gamma_sb[:])
                msc = stats.tile([128, 1], f32, tag="small")
                nc.vector.tensor_mul(msc[:], mu[:], scl[:])
                bias = singles.tile([128, 1], f32)
                nc.vector.tensor_sub(bias[:], beta_sb[:], msc[:])

                for b in range(BSH):
                    z_scr = scrp.tile([128, T], f32, tag="scr")
                    nc.scalar.activation(z_scr[:], a_all[:, b, :], AF.Tanh,
                                         bias=bias[:, 0:1], scale=scl[:, 0:1])
                    nc.scalar.activation(a_all[:, b, :], z_scr[:], AF.Exp,
                                         accum_out=Zb[:, b:b + 1])
                    nc.vector.reciprocal(rZ[:, b:b + 1], Zb[:, b:b + 1])
                    nc.vector.tensor_scalar_mul(a_all[:, b, :], a_all[:, b, :],
                                                rZ[:, b:b + 1])
                    # attn^T via DRAM round trip: [1, 2000] -> [128, 15] + [80, 1]
                    abounce = dram.tile([1, T], f32)
                    nc.sync.dma_start(out=abounce[:], in_=a_all[0:1, b, :])
                    nc.vector.memset(attnT[:, b, :], 0.0)
                    nc.sync.dma_start(
                        out=attnT[:, b, 0:NTT - 1],
                        in_=abounce[0, 0:(NTT - 1) * 128].rearrange(
                            "(tc p) -> p tc", p=128))
                    nc.sync.dma_start(
                        out=attnT[0:T - (NTT - 1) * 128, b, NTT - 1:NTT],
                        in_=abounce[0, (NTT - 1) * 128:T].rearrange(
                            "(p one) -> p one", one=1))
                    nc.scalar.sqrt(sqattnT[:, b, :], attnT[:, b, :])

                # ---- pass 2 ----
                if PHASE == "p1":
                    nc.sync.dma_start(out=out[0:1, 0:2].rearrange("a b -> b a"),
                                      in_=rZ[0:2, 0:1])
                    continue
                for b in range(BSH):
                    # PE path: kc groups of 4 -> channels [kcg*512, kcg*512+512)
                    for kcg in PE_KCG:
                        xts = []
                        for j in range(4):
                            kc = kcg * 4 + j
                            x_t = xin.tile([128, T], f32)
                            nc.sync.dma_start(
                                out=x_t[:], in_=x[b, kc * 128:(kc + 1) * 128, :])
                            xts.append(x_t)
                        acc = paccp.tile([1, 2, 512], f32)
                        for t in range(NTT):
                            tw = min(128, T - t * 128)
                            ptr = ptrp.tile([128, 4, 128], f32)
                            for j in range(4):
                                nc.tensor.transpose(
                                    ptr[0:tw, j, :],
                                    xts[j][:, t * 128:t * 128 + tw],
                                    ident[:])
                            y4 = xtp.tile([128, 4, 128], f32r, tag="xT")
                            nc.vector.tensor_scalar_mul(
                                y4[:], ptr[:], attnT[:, b, t:t + 1])
                            z4 = xtp.tile([128, 4, 128], f32r, tag="x2T")
                            nc.scalar.activation(z4[:], ptr[:], AF.Square,
                                                 scale=sqattnT[:, b, t:t + 1])
                            nc.tensor.matmul(
                                acc[:, 0, :], ones_r[:],
                                y4[:].rearrange("p a c -> p (a c)"),
                                start=(t == 0), stop=(t == NTT - 1))
                            nc.tensor.matmul(
                                acc[:, 1, :], ones_r[:],
                                z4[:].rearrange("p a c -> p (a c)"),
                                start=(t == 0), stop=(t == NTT - 1))
                        stg = stagep.tile([1, 2, 512], f32)
                        nc.scalar.copy(stg[:], acc[:])
                        m2r = stagep.tile([1, 512], f32, tag="m2r")
                        nc.vector.tensor_mul(m2r[:], stg[:, 0, :], stg[:, 0, :])
                        vor = stagep.tile([1, 512], f32, tag="vor")
                        nc.vector.tensor_sub(vor[:], stg[:, 1, :], m2r[:])
                        nc.vector.tensor_scalar_max(vor[:], vor[:], 1e-10)
                        stdr = stagep.tile([1, 512], f32, tag="stdr")
                        nc.scalar.sqrt(stdr[:], vor[:])
                        nc.sync.dma_start(
                            out=out[b:b + 1, kcg * 512:(kcg + 1) * 512],
                            in_=stg[:, 0, :])
                        nc.sync.dma_start(
                            out=out[b:b + 1, C + kcg * 512:C + (kcg + 1) * 512],
                            in_=stdr[:])

                    # DVE path: remaining kc
                    if not DVE_KC:
                        continue
                    nd = len(DVE_KC)
                    meanT = stats.tile([128, nd], f32, tag="mean")
                    sqT = stats.tile([128, nd], f32, tag="sq")
                    for ci, kc in enumerate(DVE_KC):
                        x_t = xin.tile([128, T], f32)
                        nc.sync.dma_start(out=x_t[:],
                                          in_=x[b, kc * 128:(kc + 1) * 128, :])
                        y = ypool.tile([128, T], f32)
                        nc.vector.tensor_mul(y[:], x_t[:], a_all[:, b, :])
                        nc.scalar.activation(dump[:], y[:], AF.Copy,
                                             accum_out=meanT[:, ci:ci + 1])
                        z2 = zpool.tile([128, T], f32)
                        nc.vector.tensor_mul(z2[:], y[:], x_t[:])
                        nc.scalar.activation(dump[:], z2[:], AF.Copy,
                                             accum_out=sqT[:, ci:ci + 1])
                    m2o = stats.tile([128, nd], f32, tag="m2o")
                    nc.vector.tensor_mul(m2o[:], meanT[:], meanT[:])
                    vo = stats.tile([128, nd], f32, tag="vo")
                    nc.vector.tensor_sub(vo[:], sqT[:], m2o[:])
                    nc.vector.tensor_scalar_max(vo[:], vo[:], 1e-10)
                    stdT = stats.tile([128, nd], f32, tag="std")
                    nc.scalar.sqrt(stdT[:], vo[:])
                    c0 = DVE_KC[0] * 128
                    c1 = (DVE_KC[-1] + 1) * 128
                    nc.sync.dma_start(
                        out=out[b:b + 1, c0:c1].rearrange(
                            "a (kc p) -> p (a kc)", p=128),
                        in_=meanT[:])
                    nc.sync.dma_start(
                        out=out[b:b + 1, C + c0:C + c1].rearrange(
                            "a (kc p) -> p (a kc)", p=128),
                        in_=stdT[:])
    nc.compile()
    return nc


def _get_nc(nrep=1, phase=None, pe_groups=None):
    key = (nrep, phase, pe_groups)
    if key not in _CACHE:
        _CACHE[key] = _build(nrep, phase, pe_groups)
    return _CACHE[key]


def kernel(x, conv_w, conv_b, bn_gamma, bn_beta):
    from concourse.bass_utils import run_bass_kernel_spmd

    x = np.ascontiguousarray(np.asarray(x, dtype=np.float32))
    conv_w = np.asarray(conv_w, dtype=np.float32)
    bn_gamma = np.asarray(bn_gamma, dtype=np.float32)
    bn_beta = np.asarray(bn_beta, dtype=np.float32)

    nc = _get_nc()
    in_maps = [
        {"x": x[i * BSH:(i + 1) * BSH], "conv_w": conv_w,
         "bn_gamma": bn_gamma, "bn_beta": bn_beta}
        for i in range(NCORES)
    ]
    res = run_bass_kernel_spmd(nc, in_maps, core_ids=list(range(NCORES)))
    return np.concatenate([r["out"] for r in res.results], axis=0)


# revision 11
# speedup vs baseline: 10.5923x; 1.0172x over previous
"""AttentiveStatisticsPooling Trainium2 kernel (8 NeuronCores, batch-sharded).

Reference computation (B=32, C=1536, T=2000):
    a    = einsum('bct,c->bt', x, w) + cb          # 1x1 conv -> [B,T]
    a    = BN(a)  (batch stats over all B*T, biased var)    # syncBN via AllReduce
    attn = softmax(tanh(a), axis=T)
    mean = einsum('bct,bt->bc', x, attn)
    std  = sqrt(clip(E_attn[x^2] - mean^2, 1e-10))
    out  = concat([mean, std], axis=1)             # [B, 2C]

Sharding: batch across 8 cores (4 samples each). BN batch stats are exact:
local (sum a, sum a^2) are AllReduced across cores.  conv_b cancels out of
BN(a) exactly (shift invariance), so it is unused.

V2 dataflow per core:
  pass 1: stream x[b] tiles [128c x 2000t]; DVE rounds to f32r; PE matmul
          (f32r, 1 cyc/row) with conv_w replicated across 128 output
          partitions -> a broadcast to all partitions; stats on DVE/ACT.
  mid:    AllReduce 2 scalars; BN affine folded into ACT Tanh; ACT Exp with
          accum -> softmax; attn^T obtained via a DRAM round-trip.
  pass 2: hybrid.
    PE path (kc groups of 4): PE-transpose x blocks into PSUM [t, c];
          DVE/ACT copy PSUM->SBUF as f32r (ACT applies Square for the x^2
          path); PE matmul with attn^T as stationary -> per-(b, 512ch)
          row [1,512] of weighted sums, accumulated over T in PSUM.
    DVE path (remaining kc): DVE tensor_mul y=x*attn, z=y*x; ACT Copy
          activation with accum_out reduces over T.
"""

import numpy as np

B, C, T = 32, 1536, 2000
NCORES = 8
BSH = B // NCORES          # 4 samples per core
KC = C // 128              # 12 channel chunks
NTC = 4                    # pass-1 psum T chunks
TC = T // NTC              # 500 (<=512 psum bank)
NTT = 16                   # pass-2 T chunks of 128 (last is 80)
BN_EPS = 1e-5

import os as _os

_CACHE = {}


def _build(nrep=1, phase=None, pe_groups=None, smallq=None):
    PHASE = phase if phase is not None else _os.environ.get("ASP_PHASE", "all")
    if pe_groups is None:
        pe_groups = int(_os.environ.get("ASP_PE_GROUPS", "2"))
    if smallq is None:
        smallq = _os.environ.get("ASP_SMALLQ", "gpsimd")
    PE_KCG = tuple(range(pe_groups))
    DVE_KC = tuple(range(len(PE_KCG) * 4, KC))
    import concourse.bacc as bacc
    import concourse.tile as tile
    import concourse.mybir as mybir
    from concourse.masks import make_identity

    f32 = mybir.dt.float32
    f32r = mybir.dt.float32r
    AF = mybir.ActivationFunctionType
    AX = mybir.AxisListType

    nc = bacc.Bacc("TRN2", target_bir_lowering=False, debug=False,
                   enable_asserts=True, num_devices=NCORES)
    smalldma = nc.gpsimd if smallq == "gpsimd" else nc.sync
    x = nc.dram_tensor("x", [BSH, C, T], f32, kind="ExternalInput").ap()
    w = nc.dram_tensor("conv_w", [C], f32, kind="ExternalInput").ap()
    gamma = nc.dram_tensor("bn_gamma", [1], f32, kind="ExternalInput").ap()
    beta = nc.dram_tensor("bn_beta", [1], f32, kind="ExternalInput").ap()
    out = nc.dram_tensor("out", [BSH, 2 * C], f32, kind="ExternalOutput").ap()

    with tile.TileContext(nc) as tc:
        with (
            tc.tile_pool(name="singles", bufs=1) as singles,
            tc.tile_pool(name="xin", bufs=4) as xin,
            tc.tile_pool(name="xr", bufs=3) as xrp,
            tc.tile_pool(name="xt", bufs=3) as xtp,
            tc.tile_pool(name="ypool", bufs=2) as ypool,
            tc.tile_pool(name="zpool", bufs=2) as zpool,
            tc.tile_pool(name="scr", bufs=2) as scrp,
            tc.tile_pool(name="stats", bufs=4) as stats,
            tc.tile_pool(name="stage", bufs=2) as stagep,
            tc.tile_pool(name="pa", bufs=1, space="PSUM") as pap,
            tc.tile_pool(name="ptr", bufs=2, space="PSUM") as ptrp,
            tc.tile_pool(name="pacc", bufs=1, space="PSUM") as paccp,
            tc.tile_pool(name="dram", bufs=2, space="DRAM") as dram,
        ):
            # ---- setup ----
            w_sb = singles.tile([128, KC], f32)
            nc.sync.dma_start(out=w_sb[:], in_=w.rearrange("(kc p) -> p kc", p=128))
            ones = singles.tile([128, 128], f32)
            nc.vector.memset(ones[:], 1.0)
            w_rep = singles.tile([128, KC, 128], f32r)
            for kc in range(KC):
                nc.scalar.mul(w_rep[:, kc, :], ones[:], w_sb[:, kc:kc + 1])
            ident = singles.tile([128, 128], f32)
            make_identity(nc, ident[:])
            ones_r = singles.tile([128, 1], f32r)
            nc.vector.tensor_copy(ones_r[:], ones[:, 0:1])

            gamma_sb = singles.tile([128, 1], f32)
            nc.gpsimd.dma_start(out=gamma_sb[:], in_=gamma.to_broadcast((128, 1)))
            beta_sb = singles.tile([128, 1], f32)
            nc.gpsimd.dma_start(out=beta_sb[:], in_=beta.to_broadcast((128, 1)))

            a_all = singles.tile([128, BSH, T], f32)
            attnT = singles.tile([128, BSH, NTT], f32)
            sqattnT = singles.tile([128, BSH, NTT], f32)
            sumsA = singles.tile([128, BSH], f32)
            sumsA2 = singles.tile([128, BSH], f32)
            Zb = singles.tile([128, BSH], f32)
            rZ = singles.tile([128, BSH], f32)
            dump = singles.tile([128, T], f32)

            for _rep in range(nrep):
                # ---- pass 1 ----
                if PHASE == "p2":
                    for b in range(BSH):
                        nc.vector.memset(a_all[:, b, :], 0.0005)
                    nc.vector.memset(sumsA[:], 1.0)
                    nc.vector.memset(sumsA2[:], 2.0)
                for b in range(BSH if PHASE != "p2" else 0):
                    pa = pap.tile([128, NTC, 512], f32)
                    for kc in range(KC):
                        x_t = xin.tile([128, T], f32)
                        nc.sync.dma_start(out=x_t[:],
                                          in_=x[b, kc * 128:(kc + 1) * 128, :])
                        x_r = xrp.tile([128, T], f32r)
                        nc.vector.tensor_copy(x_r[:], x_t[:])
                        for i in range(NTC):
                            nc.tensor.matmul(pa[:, i, 0:TC], w_rep[:, kc, :],
                                             x_r[:, i * TC:(i + 1) * TC],
                                             start=(kc == 0), stop=(kc == KC - 1))
                    a_b = a_all[:, b, :].rearrange("p (i n) -> p i n", i=NTC)
                    nc.scalar.copy(a_b, pa[:, :, 0:TC])
                    nc.vector.reduce_sum(sumsA[:, b:b + 1], a_all[:, b, :], axis=AX.X)
                    sq_scr = scrp.tile([128, T], f32, tag="scr")
                    nc.scalar.activation(sq_scr[:], a_all[:, b, :], AF.Square,
                                         accum_out=sumsA2[:, b:b + 1])

                # ---- mid: BN stats allreduce, attn = softmax(tanh(bn(a))) ----
                if PHASE == "p1only":
                    nc.sync.dma_start(out=out[0:1, 0:2].rearrange("a b -> b a"),
                                      in_=sumsA[0:2, 0:1])
                    continue
                totA = stats.tile([128, 1], f32, tag="small")
                nc.vector.reduce_sum(totA[:], sumsA[:], axis=AX.X)
                totA2 = stats.tile([128, 1], f32, tag="small")
                nc.vector.reduce_sum(totA2[:], sumsA2[:], axis=AX.X)

                cc_in = dram.tile([1, 2], f32)
                cc_out = dram.tile([1, 2], f32)
                smalldma.dma_start(out=cc_in[0:1, 0:1], in_=totA[0:1, 0:1])
                smalldma.dma_start(out=cc_in[0:1, 1:2], in_=totA2[0:1, 0:1])
                nc.gpsimd.collective_compute(
                    "AllReduce", mybir.AluOpType.add,
                    replica_groups=[list(range(NCORES))],
                    ins=[cc_in.opt()], outs=[cc_out.opt()])
                g = stats.tile([128, 2], f32, tag="small2")
                nc.gpsimd.dma_start(out=g[:], in_=cc_out.to_broadcast((128, 2)))

                inv_n = 1.0 / float(B * T)
                mu = stats.tile([128, 1], f32, tag="small")
                nc.vector.tensor_scalar_mul(mu[:], g[:, 0:1], inv_n)
                ex2 = stats.tile([128, 1], f32, tag="small")
                nc.vector.tensor_scalar_mul(ex2[:], g[:, 1:2], inv_n)
                m2 = stats.tile([128, 1], f32, tag="small")
                nc.vector.tensor_mul(m2[:], mu[:], mu[:])
                var = stats.tile([128, 1], f32, tag="small")
                nc.vector.tensor_sub(var[:], ex2[:], m2[:])
                vep = stats.tile([128, 1], f32, tag="small")
                nc.vector.tensor_scalar_add(vep[:], var[:], BN_EPS)
                sd = stats.tile([128, 1], f32, tag="small")
                nc.scalar.sqrt(sd[:], vep[:])
                rstd = stats.tile([128, 1], f32, tag="small")
                nc.vector.reciprocal(rstd[:], sd[:])
                scl = singles.tile([128, 1], f32)
                nc.vector.tensor_mul(scl[:], rstd[:], gamma_sb[:])
                msc = stats.tile([128, 1], f32, tag="small")
                nc.vector.tensor_mul(msc[:], mu[:], scl[:])
                bias = singles.tile([128, 1], f32)
                nc.vector.tensor_sub(bias[:], beta_sb[:], msc[:])

                for b in range(BSH):
                    z_scr = scrp.tile([128, T], f32, tag="scr")
                    nc.scalar.activation(z_scr[:], a_all[:, b, :], AF.Tanh,
                                         bias=bias[:, 0:1], scale=scl[:, 0:1])
                    nc.scalar.activation(a_all[:, b, :], z_scr[:], AF.Exp,
                                         accum_out=Zb[:, b:b + 1])
                    nc.vector.reciprocal(rZ[:, b:b + 1], Zb[:, b:b + 1])
                    nc.vector.tensor_scalar_mul(a_all[:, b, :], a_all[:, b, :],
                                                rZ[:, b:b + 1])
                    # attn^T via DRAM round trip: [1, 2000] -> [128, 15] + [80, 1]
                    abounce = dram.tile([1, T], f32)
                    smalldma.dma_start(out=abounce[:], in_=a_all[0:1, b, :])
                    nc.vector.memset(attnT[:, b, :], 0.0)
                    smalldma.dma_start(
                        out=attnT[:, b, 0:NTT - 1],
                        in_=abounce[0, 0:(NTT - 1) * 128].rearrange(
                            "(tc p) -> p tc", p=128))
                    smalldma.dma_start(
                        out=attnT[0:T - (NTT - 1) * 128, b, NTT - 1:NTT],
                        in_=abounce[0, (NTT - 1) * 128:T].rearrange(
                            "(p one) -> p one", one=1))
                    nc.scalar.sqrt(sqattnT[:, b, :], attnT[:, b, :])

                # ---- pass 2 ----
                if PHASE == "p1":
                    nc.sync.dma_start(out=out[0:1, 0:2].rearrange("a b -> b a"),
                                      in_=rZ[0:2, 0:1])
                    continue
                for b in range(BSH):
                    # PE path: kc groups of 4 -> channels [kcg*512, kcg*512+512)
                    for kcg in PE_KCG:
                        xts = []
                        for j in range(4):
                            kc = kcg * 4 + j
                            x_t = xin.tile([128, T], f32)
                            nc.sync.dma_start(
                                out=x_t[:], in_=x[b, kc * 128:(kc + 1) * 128, :])
                            xts.append(x_t)
                        acc = paccp.tile([1, 2, 512], f32)
                        for t in range(NTT):
                            tw = min(128, T - t * 128)
                            ptr = ptrp.tile([128, 4, 128], f32)
                            for j in range(4):
                                nc.tensor.transpose(
                                    ptr[0:tw, j, :],
                                    xts[j][:, t * 128:t * 128 + tw],
                                    ident[:])
                            y4 = xtp.tile([128, 4, 128], f32r, tag="xT")
                            nc.vector.tensor_scalar_mul(
                                y4[:], ptr[:], attnT[:, b, t:t + 1])
                            z4 = xtp.tile([128, 4, 128], f32r, tag="x2T")
                            nc.scalar.activation(z4[:], ptr[:], AF.Square,
                                                 scale=sqattnT[:, b, t:t + 1])
                            nc.tensor.matmul(
                                acc[:, 0, :], ones_r[:],
                                y4[:].rearrange("p a c -> p (a c)"),
                                start=(t == 0), stop=(t == NTT - 1))
                            nc.tensor.matmul(
                                acc[:, 1, :], ones_r[:],
                                z4[:].rearrange("p a c -> p (a c)"),
                                start=(t == 0), stop=(t == NTT - 1))
                        stg = stagep.tile([1, 2, 512], f32)
                        nc.scalar.copy(stg[:], acc[:])
                        m2r = stagep.tile([1, 512], f32, tag="m2r")
                        nc.vector.tensor_mul(m2r[:], stg[:, 0, :], stg[:, 0, :])
                        vor = stagep.tile([1, 512], f32, tag="vor")
                        nc.vector.tensor_sub(vor[:], stg[:, 1, :], m2r[:])
                        nc.vector.tensor_scalar_max(vor[:], vor[:], 1e-10)
                        stdr = stagep.tile([1, 512], f32, tag="stdr")
                        nc.scalar.sqrt(stdr[:], vor[:])
                        smalldma.dma_start(
                            out=out[b:b + 1, kcg * 512:(kcg + 1) * 512],
                            in_=stg[:, 0, :])
                        smalldma.dma_start(
                            out=out[b:b + 1, C + kcg * 512:C + (kcg + 1) * 512],
                            in_=stdr[:])

                    # DVE path: remaining kc
                    if not DVE_KC:
                        continue
                    nd = len(DVE_KC)
                    meanT = stats.tile([128, nd], f32, tag="mean")
                    sqT = stats.tile([128, nd], f32, tag="sq")
                    for ci, kc in enumerate(DVE_KC):
                        x_t = xin.tile([128, T], f32)
                        nc.sync.dma_start(out=x_t[:],
                                          in_=x[b, kc * 128:(kc + 1) * 128, :])
                        y = ypool.tile([128, T], f32)
                        nc.vector.tensor_mul(y[:], x_t[:], a_all[:, b, :])
                        nc.scalar.activation(dump[:], y[:], AF.Copy,
                                             accum_out=meanT[:, ci:ci + 1])
                        z2 = zpool.tile([128, T], f32)
                        nc.vector.tensor_mul(z2[:], y[:], x_t[:])
                        nc.scalar.activation(dump[:], z2[:], AF.Copy,
                                             accum_out=sqT[:, ci:ci + 1])
                    m2o = stats.tile([128, nd], f32, tag="m2o")
                    nc.vector.tensor_mul(m2o[:], meanT[:], meanT[:])
                    vo = stats.tile([128, nd], f32, tag="vo")
                    nc.vector.tensor_sub(vo[:], sqT[:], m2o[:])
                    nc.vector.tensor_scalar_max(vo[:], vo[:], 1e-10)
                    stdT = stats.tile([128, nd], f32, tag="std")
                    nc.scalar.sqrt(stdT[:], vo[:])
                    c0 = DVE_KC[0] * 128
                    c1 = (DVE_KC[-1] + 1) * 128
                    smalldma.dma_start(
                        out=out[b:b + 1, c0:c1].rearrange(
                            "a (kc p) -> p (a kc)", p=128),
                        in_=meanT[:])
                    smalldma.dma_start(
                        out=out[b:b + 1, C + c0:C + c1].rearrange(
                            "a (kc p) -> p (a kc)", p=128),
                        in_=stdT[:])
    nc.compile()
    return nc


def _get_nc(nrep=1, phase=None, pe_groups=None, smallq=None):
    key = (nrep, phase, pe_groups, smallq)
    if key not in _CACHE:
        _CACHE[key] = _build(nrep, phase, pe_groups, smallq)
    return _CACHE[key]


def kernel(x, conv_w, conv_b, bn_gamma, bn_beta):
    from concourse.bass_utils import run_bass_kernel_spmd

    x = np.ascontiguousarray(np.asarray(x, dtype=np.float32))
    conv_w = np.asarray(conv_w, dtype=np.float32)
    bn_gamma = np.asarray(bn_gamma, dtype=np.float32)
    bn_beta = np.asarray(bn_beta, dtype=np.float32)

    nc = _get_nc()
    in_maps = [
        {"x": x[i * BSH:(i + 1) * BSH], "conv_w": conv_w,
         "bn_gamma": bn_gamma, "bn_beta": bn_beta}
        for i in range(NCORES)
    ]
    res = run_bass_kernel_spmd(nc, in_maps, core_ids=list(range(NCORES)))
    return np.concatenate([r["out"] for r in res.results], axis=0)
